# revision 2
# baseline (speedup 1.0000x reference)
"""Trilinear 3D grid-encoding lookup on 8 TRN2 NeuronCores — v4 (ap_gather).

The baseline was GpSimd-bound: dma_gather descriptor generation costs
~7.9ns/point on the Q7s (2.03ms/core).  v4 replaces it with ap_gather —
an SBUF->SBUF free-dim gather where each of the 8 Q7 cores gathers with
its own indices — plus a PE matmul to reduce the 8 weighted corners.

Layout (per NeuronCore n):
  - 64 global windows of 32768 cells (cell>>15); Q7 core k owns window
    w=8n+k.  Points sorted by cell, binned per window, padded to S=32768
    slots; slot j of core k lives at idx partition 16k+(j%16), col j//16.
  - Table tile [128, 32768, 2] f16: partition 16k+2co+h holds, for each
    local cell e, channels (2h, 2h+1) of corner co (corner-expanded with
    edge clamping).  128KB/partition, SBUF-resident.
  - ap_gather call (J=4096 slots): out [128, J, 2]; point (k,j)'s 32
    corner values live at partitions 16k..16k+15, col j.  ap_gather's
    true throughput is ~27ns/idx/Q7-core (~3.4ns/point) — 2.3x faster
    than dma_gather's descriptor generation (7.9ns/point).
  - Host-computed trilinear weights wi[16k+2co+h, j]; DVE multiplies
    gathered values by weights in place (broadcast over the f16 pair).
  - PE matmul with constant block-ones lhsT [128,16] reduces the 8
    corners per channel: psum[2k+h, (j,s)] = ch(2h+s) of point (k,j).
  - DVE/ACT drain psum -> f16 SBUF -> one DMA per call to DRAM.
  - Host inverse-permutes to input order.
"""
import numpy as np

import concourse.bacc as bacc
import concourse.bass as bass
import concourse.mybir as mybir
from concourse.bass_utils import run_bass_kernel_spmd
from concourse.tile import TileContext

NBINS = 128
OUTC = 4
CORES = 8                 # NeuronCores
QC = 8                    # Q7 cores per NeuronCore
NWIN = CORES * QC         # 64 global windows
WCELLS = NBINS ** 3 // NWIN   # 32768 cells per window
S = 32768                 # point slots per window
J = 4096                  # slots per ap_gather call (per Q7 core)
CALLS = S // J            # 16
CHUNK = 512               # matmul free cols per psum bank

F32 = mybir.dt.float32
F16 = mybir.dt.float16
I16 = mybir.dt.int16
OP = mybir.AluOpType

LAST_RESULT = None
_NC_CACHE = None


TCH = 8            # table DMA chunks
TQ = WCELLS // TCH


def _build():
    nc = bacc.Bacc(None, target_bir_lowering=False)
    tabi = nc.dram_tensor("tabi", [128, WCELLS, 2], F16, kind="ExternalInput")
    wi = nc.dram_tensor("wi", [128, S], F16, kind="ExternalInput")
    idxi = nc.dram_tensor("idxi", [128, S // 16], I16, kind="ExternalInput")
    onesi = nc.dram_tensor("onesi", [128, 16], F16, kind="ExternalInput")
    outo = nc.dram_tensor("outo", [16, 2 * S], F16, kind="ExternalOutput")

    with TileContext(nc) as tc:
        with tc.tile_pool(name="tp", bufs=1) as tpool, \
             tc.tile_pool(name="ps", bufs=8, space="PSUM") as ppool:
            tabs = tpool.tile([128, WCELLS, 2], F16, name="tabs")
            idxt = tpool.tile([128, S // 16], I16, name="idxt")
            ones = tpool.tile([128, 16], F16, name="ones")
            # everything the Pool engine reads goes on ONE queue (sync) so a
            # single pool-side "touch" establishes the whole watermark with
            # one inline semaphore wait.  A Pool instruction with >1 sem wait
            # gets a separate EventSemaphore, which costs ~50us on the Pool
            # NX regardless of whether the condition is already satisfied.
            nc.sync.dma_start(out=ones[:], in_=onesi[:])
            nc.sync.dma_start(out=idxt[:], in_=idxi[:])
            for t in range(TCH):
                nc.sync.dma_start(out=tabs[:, t * TQ:(t + 1) * TQ, :],
                                  in_=tabi[:, t * TQ:(t + 1) * TQ, :])

            ga = [tpool.tile([128, J, 2], F16, name=f"ga{i}") for i in (0, 1)]
            wt = [tpool.tile([128, J], F16, name="wt0")] * 2
            ob = [tpool.tile([16, 2 * J], F16, name=f"ob{i}") for i in (0, 1)]
            touch = tpool.tile([128, 4], F16, name="touch")
            nc.gpsimd.tensor_copy(out=touch[:].unsqueeze(-1),
                                  in_=tabs[:, WCELLS - 4:WCELLS, 0:1])

            for c in range(CALLS):
                cur = ga[c % 2]
                w_c, o_c = wt[c % 2], ob[c % 2]
                nc.scalar.dma_start(out=w_c[:], in_=wi[:, c * J:(c + 1) * J])
                nc.gpsimd.ap_gather(
                    out_ap=cur[:], in_ap=tabs[:],
                    idxs_ap=idxt[:, c * (J // 16):(c + 1) * (J // 16)],
                    channels=128, num_elems=WCELLS, d=2, num_idxs=J)
                nc.vector.tensor_tensor(
                    out=cur[:], in0=cur[:],
                    in1=w_c[:].unsqueeze(-1).to_broadcast([128, J, 2]),
                    op=OP.mult)
                for t in range(2 * J // CHUNK):      # 8 chunks of 512
                    ps = ppool.tile([16, CHUNK], F32, name="ps")
                    nc.tensor.matmul(
                        ps[:], ones[:],
                        cur[:, t * (CHUNK // 2):(t + 1) * (CHUNK // 2), :])
                    if t % 2 == 0:
                        nc.vector.tensor_copy(
                            out=o_c[:, t * CHUNK:(t + 1) * CHUNK], in_=ps[:])
                    else:
                        nc.scalar.activation(
                            out=o_c[:, t * CHUNK:(t + 1) * CHUNK], in_=ps[:],
                            func=mybir.ActivationFunctionType.Copy)
                nc.sync.dma_start(out=outo[:, c * 2 * J:(c + 1) * 2 * J],
                                  in_=o_c[:])
    nc.compile()
    return nc


def _build_table(grid: np.ndarray) -> np.ndarray:
    """[NWIN, 16, WCELLS, 2] f16: [w, 2*co+h, e, s] corner-expanded."""
    gp = np.pad(np.asarray(grid, np.float32),
                ((0, 1), (0, 1), (0, 1), (0, 0)), mode="edge").astype(np.float16)
    sw = np.lib.stride_tricks.sliding_window_view(gp, (2, 2, 2), axis=(0, 1, 2))
    # sw: [128,128,128, 4ch, 2dx, 2dy, 2dz]
    e = sw.transpose(0, 1, 2, 4, 5, 6, 3)          # [ix,iy,iz, dx,dy,dz, ch]
    e = e.reshape(NWIN, WCELLS, QC, 2, 2)          # [w, e, co, h, s]
    tab = e.transpose(0, 2, 3, 1, 4)               # [w, co, h, e, s]
    return np.ascontiguousarray(tab, dtype=np.float16).reshape(
        NWIN, 16, WCELLS, 2)


def kernel(x: np.ndarray, grid: np.ndarray) -> np.ndarray:
    global LAST_RESULT, _NC_CACHE
    x = np.asarray(x, dtype=np.float32)
    n = x.shape[0]

    p = np.clip(x * np.float32(NBINS), 0.0, np.float32(NBINS - 1))
    i0 = np.floor(p).astype(np.int32)
    f = p - i0.astype(np.float32)
    cell = (i0[:, 0] * NBINS + i0[:, 1]) * NBINS + i0[:, 2]
    win = cell >> 15
    local = (cell & (WCELLS - 1)).astype(np.int16)

    # 8 weights per point, co = dx*4+dy*2+dz
    wx = np.stack([1.0 - f[:, 0], f[:, 0]], 1)     # [N,2]
    wy = np.stack([1.0 - f[:, 1], f[:, 1]], 1)
    wz = np.stack([1.0 - f[:, 2], f[:, 2]], 1)
    w8 = (wx[:, :, None, None] * wy[:, None, :, None] *
          wz[:, None, None, :]).reshape(n, 8).astype(np.float16)

    order = np.argsort(win, kind="stable")
    counts = np.bincount(win, minlength=NWIN)
    if counts.max() > S:
        raise RuntimeError(f"window overflow: {counts.max()} > {S}")
    offs = np.zeros(NWIN + 1, dtype=np.int64)
    np.cumsum(counts, out=offs[1:])
    assign = np.empty((NWIN, S), dtype=np.int64)
    for w in range(NWIN):
        pts = order[offs[w]:offs[w + 1]]
        pad = pts[0] if len(pts) else order[0]
        assign[w, :len(pts)] = pts
        assign[w, len(pts):] = pad

    tab_all = _build_table(grid)                    # [NWIN, 16, WCELLS, 2]

    onesi = np.zeros((128, 16), dtype=np.float16)
    for k in range(QC):
        for co in range(8):
            for h in range(2):
                onesi[16 * k + 2 * co + h, 2 * k + h] = 1.0

    in_maps = []
    for nn in range(CORES):
        sel = assign[QC * nn:QC * nn + QC]          # [8, S]
        # idx: [16k + j%16, j//16]
        loc = local[sel]                            # [8, S]
        idxn = np.ascontiguousarray(
            loc.reshape(QC, S // 16, 16).transpose(0, 2, 1).reshape(128, S // 16))
        # weights: [16k + 2co + h, j]
        wn = w8[sel]                                # [8, S, 8]
        wn = wn.transpose(0, 2, 1)                  # [8k, 8co, S]
        wn = np.repeat(wn[:, :, None, :], 2, axis=2)  # [8k, 8co, 2h, S]
        wn = np.ascontiguousarray(wn.reshape(128, S))
        tabn = np.ascontiguousarray(
            tab_all[QC * nn:QC * nn + QC].reshape(128, WCELLS, 2))
        in_maps.append({"tabi": tabn, "wi": wn, "idxi": idxn, "onesi": onesi})

    if _NC_CACHE is None:
        _NC_CACHE = _build()
    res = run_bass_kernel_spmd(_NC_CACHE, in_maps, core_ids=list(range(CORES)))
    LAST_RESULT = res

    outp = np.empty((n, OUTC), dtype=np.float32)
    for nn in range(CORES):
        o = np.asarray(res.results[nn]["outo"], dtype=np.float32)
        vals = o.reshape(QC, 2, S, 2).transpose(0, 2, 1, 3).reshape(QC, S, OUTC)
        for k in range(QC):
            w = QC * nn + k
            cnt = int(counts[w])
            outp[assign[w, :cnt]] = vals[k, :cnt]
    return outp


# revision 3
# speedup vs baseline: 1.0348x; 1.0348x over previous
"""Trilinear 3D grid-encoding lookup on 8 TRN2 NeuronCores — v4 (ap_gather).

The baseline was GpSimd-bound: dma_gather descriptor generation costs
~7.9ns/point on the Q7s (2.03ms/core).  v4 replaces it with ap_gather —
an SBUF->SBUF free-dim gather where each of the 8 Q7 cores gathers with
its own indices — plus a PE matmul to reduce the 8 weighted corners.

Layout (per NeuronCore n):
  - 64 global windows of 32768 cells (cell>>15); Q7 core k owns window
    w=8n+k.  Points sorted by cell, binned per window, padded to S=32768
    slots; slot j of core k lives at idx partition 16k+(j%16), col j//16.
  - Table tile [128, 32768, 2] f16: partition 16k+2co+h holds, for each
    local cell e, channels (2h, 2h+1) of corner co (corner-expanded with
    edge clamping).  128KB/partition, SBUF-resident.
  - ap_gather call (J=4096 slots): out [128, J, 2]; point (k,j)'s 32
    corner values live at partitions 16k..16k+15, col j.  ap_gather's
    true throughput is ~27ns/idx/Q7-core (~3.4ns/point) — 2.3x faster
    than dma_gather's descriptor generation (7.9ns/point).
  - Host-computed trilinear weights wi[16k+2co+h, j]; DVE multiplies
    gathered values by weights in place (broadcast over the f16 pair).
  - PE matmul with constant block-ones lhsT [128,16] reduces the 8
    corners per channel: psum[2k+h, (j,s)] = ch(2h+s) of point (k,j).
  - DVE/ACT drain psum -> f16 SBUF -> one DMA per call to DRAM.
  - Host inverse-permutes to input order.
"""
import numpy as np

import concourse.bacc as bacc
import concourse.bass as bass
import concourse.mybir as mybir
from concourse.bass_utils import run_bass_kernel_spmd
from concourse.tile import TileContext

NBINS = 128
OUTC = 4
CORES = 8                 # NeuronCores
QC = 8                    # Q7 cores per NeuronCore
NWIN = CORES * QC         # 64 global windows
WCELLS = NBINS ** 3 // NWIN   # 32768 cells per window
S = 32768                 # point slots per window
J = 4096                  # slots per ap_gather call (per Q7 core)
CALLS = S // J            # 16
CHUNK = 512               # matmul free cols per psum bank

F32 = mybir.dt.float32
F16 = mybir.dt.float16
I16 = mybir.dt.int16
OP = mybir.AluOpType

LAST_RESULT = None
_NC_CACHE = None


TCH = 8            # table DMA chunks
TQ = WCELLS // TCH


def _build():
    nc = bacc.Bacc(None, target_bir_lowering=False)
    tabi = nc.dram_tensor("tabi", [128, WCELLS, 2], F16, kind="ExternalInput")
    wi = nc.dram_tensor("wi", [128, S], F16, kind="ExternalInput")
    idxi = nc.dram_tensor("idxi", [128, S // 16], I16, kind="ExternalInput")
    onesi = nc.dram_tensor("onesi", [128, 16], F16, kind="ExternalInput")
    outo = nc.dram_tensor("outo", [16, 2 * S], F16, kind="ExternalOutput")

    with TileContext(nc) as tc:
        with tc.tile_pool(name="tp", bufs=1) as tpool, \
             tc.tile_pool(name="ps", bufs=8, space="PSUM") as ppool:
            tabs = tpool.tile([128, WCELLS, 2], F16, name="tabs")
            idxt = tpool.tile([128, S // 16], I16, name="idxt")
            ones = tpool.tile([128, 16], F16, name="ones")
            # everything the Pool engine reads goes on ONE queue (sync) so a
            # single pool-side "touch" establishes the whole watermark with
            # one inline semaphore wait.  A Pool instruction with >1 sem wait
            # gets a separate EventSemaphore, which costs ~50us on the Pool
            # NX regardless of whether the condition is already satisfied.
            nc.sync.dma_start(out=ones[:], in_=onesi[:])
            nc.sync.dma_start(out=idxt[:], in_=idxi[:])
            # table split across both HWDGE queues to halve the load ramp
            for t in range(TCH):
                eng = nc.sync if t < TCH // 2 else nc.scalar
                eng.dma_start(out=tabs[:, t * TQ:(t + 1) * TQ, :],
                              in_=tabi[:, t * TQ:(t + 1) * TQ, :])

            ga = [tpool.tile([128, J, 2], F16, name=f"ga{i}") for i in (0, 1)]
            wt = [tpool.tile([128, J], F16, name="wt0")] * 2
            ob = [tpool.tile([16, 2 * J], F16, name=f"ob{i}") for i in (0, 1)]
            touch = tpool.tile([128, 4], F16, name="touch")
            # one pool touch per DMA queue: each carries a single inline
            # semaphore wait (a multi-wait Pool EventSemaphore costs ~55us)
            nc.gpsimd.tensor_copy(out=touch[:].unsqueeze(-1),
                                  in_=tabs[:, TCH // 2 * TQ - 4:TCH // 2 * TQ, 0:1])
            nc.gpsimd.tensor_copy(out=touch[:].unsqueeze(-1),
                                  in_=tabs[:, WCELLS - 4:WCELLS, 0:1])

            # tail calls shrink so the last gather's hidden Q7 write-drain
            # (~27ns/idx) exposes less in the end barrier
            specs = [(i * J, J) for i in range(CALLS - 1)]
            specs += [(S - J, J // 2), (S - J // 2, J // 4), (S - J // 4, J // 4)]

            for c, (st, sz) in enumerate(specs):
                cur = ga[c % 2]
                w_c, o_c = wt[c % 2], ob[c % 2]
                nc.scalar.dma_start(out=w_c[:, 0:sz], in_=wi[:, st:st + sz])
                nc.gpsimd.ap_gather(
                    out_ap=cur[:, 0:sz, :], in_ap=tabs[:],
                    idxs_ap=idxt[:, st // 16:(st + sz) // 16],
                    channels=128, num_elems=WCELLS, d=2, num_idxs=sz)
                nc.vector.tensor_tensor(
                    out=cur[:, 0:sz, :], in0=cur[:, 0:sz, :],
                    in1=w_c[:, 0:sz].unsqueeze(-1).to_broadcast([128, sz, 2]),
                    op=OP.mult)
                for t in range(2 * sz // CHUNK):
                    ps = ppool.tile([16, CHUNK], F32, name="ps")
                    nc.tensor.matmul(
                        ps[:], ones[:],
                        cur[:, t * (CHUNK // 2):(t + 1) * (CHUNK // 2), :])
                    if t % 2 == 0:
                        nc.vector.tensor_copy(
                            out=o_c[:, t * CHUNK:(t + 1) * CHUNK], in_=ps[:])
                    else:
                        nc.scalar.activation(
                            out=o_c[:, t * CHUNK:(t + 1) * CHUNK], in_=ps[:],
                            func=mybir.ActivationFunctionType.Copy)
                nc.sync.dma_start(out=outo[:, 2 * st:2 * (st + sz)],
                                  in_=o_c[:, 0:2 * sz])
    nc.compile()
    return nc


def _build_table(grid: np.ndarray) -> np.ndarray:
    """[NWIN, 16, WCELLS, 2] f16: [w, 2*co+h, e, s] corner-expanded."""
    gp = np.pad(np.asarray(grid, np.float32),
                ((0, 1), (0, 1), (0, 1), (0, 0)), mode="edge").astype(np.float16)
    sw = np.lib.stride_tricks.sliding_window_view(gp, (2, 2, 2), axis=(0, 1, 2))
    # sw: [128,128,128, 4ch, 2dx, 2dy, 2dz]
    e = sw.transpose(0, 1, 2, 4, 5, 6, 3)          # [ix,iy,iz, dx,dy,dz, ch]
    e = e.reshape(NWIN, WCELLS, QC, 2, 2)          # [w, e, co, h, s]
    tab = e.transpose(0, 2, 3, 1, 4)               # [w, co, h, e, s]
    return np.ascontiguousarray(tab, dtype=np.float16).reshape(
        NWIN, 16, WCELLS, 2)


def kernel(x: np.ndarray, grid: np.ndarray) -> np.ndarray:
    global LAST_RESULT, _NC_CACHE
    x = np.asarray(x, dtype=np.float32)
    n = x.shape[0]

    p = np.clip(x * np.float32(NBINS), 0.0, np.float32(NBINS - 1))
    i0 = np.floor(p).astype(np.int32)
    f = p - i0.astype(np.float32)
    cell = (i0[:, 0] * NBINS + i0[:, 1]) * NBINS + i0[:, 2]
    win = cell >> 15
    local = (cell & (WCELLS - 1)).astype(np.int16)

    # 8 weights per point, co = dx*4+dy*2+dz
    wx = np.stack([1.0 - f[:, 0], f[:, 0]], 1)     # [N,2]
    wy = np.stack([1.0 - f[:, 1], f[:, 1]], 1)
    wz = np.stack([1.0 - f[:, 2], f[:, 2]], 1)
    w8 = (wx[:, :, None, None] * wy[:, None, :, None] *
          wz[:, None, None, :]).reshape(n, 8).astype(np.float16)

    order = np.argsort(win, kind="stable")
    counts = np.bincount(win, minlength=NWIN)
    if counts.max() > S:
        raise RuntimeError(f"window overflow: {counts.max()} > {S}")
    offs = np.zeros(NWIN + 1, dtype=np.int64)
    np.cumsum(counts, out=offs[1:])
    assign = np.empty((NWIN, S), dtype=np.int64)
    for w in range(NWIN):
        pts = order[offs[w]:offs[w + 1]]
        pad = pts[0] if len(pts) else order[0]
        assign[w, :len(pts)] = pts
        assign[w, len(pts):] = pad

    tab_all = _build_table(grid)                    # [NWIN, 16, WCELLS, 2]

    onesi = np.zeros((128, 16), dtype=np.float16)
    for k in range(QC):
        for co in range(8):
            for h in range(2):
                onesi[16 * k + 2 * co + h, 2 * k + h] = 1.0

    in_maps = []
    for nn in range(CORES):
        sel = assign[QC * nn:QC * nn + QC]          # [8, S]
        # idx: [16k + j%16, j//16]
        loc = local[sel]                            # [8, S]
        idxn = np.ascontiguousarray(
            loc.reshape(QC, S // 16, 16).transpose(0, 2, 1).reshape(128, S // 16))
        # weights: [16k + 2co + h, j]
        wn = w8[sel]                                # [8, S, 8]
        wn = wn.transpose(0, 2, 1)                  # [8k, 8co, S]
        wn = np.repeat(wn[:, :, None, :], 2, axis=2)  # [8k, 8co, 2h, S]
        wn = np.ascontiguousarray(wn.reshape(128, S))
        tabn = np.ascontiguousarray(
            tab_all[QC * nn:QC * nn + QC].reshape(128, WCELLS, 2))
        in_maps.append({"tabi": tabn, "wi": wn, "idxi": idxn, "onesi": onesi})

    if _NC_CACHE is None:
        _NC_CACHE = _build()
    res = run_bass_kernel_spmd(_NC_CACHE, in_maps, core_ids=list(range(CORES)))
    LAST_RESULT = res

    outp = np.empty((n, OUTC), dtype=np.float32)
    for nn in range(CORES):
        o = np.asarray(res.results[nn]["outo"], dtype=np.float32)
        vals = o.reshape(QC, 2, S, 2).transpose(0, 2, 1, 3).reshape(QC, S, OUTC)
        for k in range(QC):
            w = QC * nn + k
            cnt = int(counts[w])
            outp[assign[w, :cnt]] = vals[k, :cnt]
    return outp


# revision 4
# speedup vs baseline: 1.0370x; 1.0021x over previous
"""Trilinear 3D grid-encoding lookup on 8 TRN2 NeuronCores — v4 (ap_gather).

The baseline was GpSimd-bound: dma_gather descriptor generation costs
~7.9ns/point on the Q7s (2.03ms/core).  v4 replaces it with ap_gather —
an SBUF->SBUF free-dim gather where each of the 8 Q7 cores gathers with
its own indices — plus a PE matmul to reduce the 8 weighted corners.

Layout (per NeuronCore n):
  - 64 global windows of 32768 cells (cell>>15); Q7 core k owns window
    w=8n+k.  Points sorted by cell, binned per window, padded to S=32768
    slots; slot j of core k lives at idx partition 16k+(j%16), col j//16.
  - Table tile [128, 32768, 2] f16: partition 16k+2co+h holds, for each
    local cell e, channels (2h, 2h+1) of corner co (corner-expanded with
    edge clamping).  128KB/partition, SBUF-resident.
  - ap_gather call (J=4096 slots): out [128, J, 2]; point (k,j)'s 32
    corner values live at partitions 16k..16k+15, col j.  ap_gather's
    true throughput is ~27ns/idx/Q7-core (~3.4ns/point) — 2.3x faster
    than dma_gather's descriptor generation (7.9ns/point).
  - Host-computed trilinear weights wi[16k+2co+h, j]; DVE multiplies
    gathered values by weights in place (broadcast over the f16 pair).
  - PE matmul with constant block-ones lhsT [128,16] reduces the 8
    corners per channel: psum[2k+h, (j,s)] = ch(2h+s) of point (k,j).
  - DVE/ACT drain psum -> f16 SBUF -> one DMA per call to DRAM.
  - Host inverse-permutes to input order.
"""
import numpy as np

import concourse.bacc as bacc
import concourse.bass as bass
import concourse.mybir as mybir
from concourse.bass_utils import run_bass_kernel_spmd
from concourse.tile import TileContext

NBINS = 128
OUTC = 4
CORES = 8                 # NeuronCores
QC = 8                    # Q7 cores per NeuronCore
NWIN = CORES * QC         # 64 global windows
WCELLS = NBINS ** 3 // NWIN   # 32768 cells per window
S = 32768                 # point slots per window
J = 4096                  # slots per ap_gather call (per Q7 core)
CALLS = S // J            # 16
CHUNK = 512               # matmul free cols per psum bank

F32 = mybir.dt.float32
F16 = mybir.dt.float16
I16 = mybir.dt.int16
OP = mybir.AluOpType

LAST_RESULT = None
_NC_CACHE = None


TCH = 8            # table DMA chunks
TQ = WCELLS // TCH


def _build(s_eff=S):
    nc = bacc.Bacc(None, target_bir_lowering=False)
    tabi = nc.dram_tensor("tabi", [128, WCELLS, 2], F16, kind="ExternalInput")
    wi = nc.dram_tensor("wi", [128, S], F16, kind="ExternalInput")
    idxi = nc.dram_tensor("idxi", [128, S // 16], I16, kind="ExternalInput")
    onesi = nc.dram_tensor("onesi", [128, 16], F16, kind="ExternalInput")
    outo = nc.dram_tensor("outo", [16, 2 * S], F16, kind="ExternalOutput")

    with TileContext(nc) as tc:
        with tc.tile_pool(name="tp", bufs=1) as tpool, \
             tc.tile_pool(name="ps", bufs=8, space="PSUM") as ppool:
            tabs = tpool.tile([128, WCELLS, 2], F16, name="tabs")
            idxt = tpool.tile([128, S // 16], I16, name="idxt")
            ones = tpool.tile([128, 16], F16, name="ones")
            # everything the Pool engine reads goes on ONE queue (sync) so a
            # single pool-side "touch" establishes the whole watermark with
            # one inline semaphore wait.  A Pool instruction with >1 sem wait
            # gets a separate EventSemaphore, which costs ~50us on the Pool
            # NX regardless of whether the condition is already satisfied.
            nc.sync.dma_start(out=ones[:], in_=onesi[:])
            nc.sync.dma_start(out=idxt[:], in_=idxi[:])
            # table split across both HWDGE queues to halve the load ramp
            for t in range(TCH):
                eng = nc.sync if t < TCH // 2 else nc.scalar
                eng.dma_start(out=tabs[:, t * TQ:(t + 1) * TQ, :],
                              in_=tabi[:, t * TQ:(t + 1) * TQ, :])

            ga = [tpool.tile([128, J, 2], F16, name=f"ga{i}") for i in (0, 1)]
            wt = [tpool.tile([128, J], F16, name="wt0")] * 2
            ob = [tpool.tile([16, 2 * J], F16, name=f"ob{i}") for i in (0, 1)]
            touch = tpool.tile([128, 4], F16, name="touch")
            # one pool touch per DMA queue: each carries a single inline
            # semaphore wait (a multi-wait Pool EventSemaphore costs ~55us)
            nc.gpsimd.tensor_copy(out=touch[:].unsqueeze(-1),
                                  in_=tabs[:, TCH // 2 * TQ - 4:TCH // 2 * TQ, 0:1])
            nc.gpsimd.tensor_copy(out=touch[:].unsqueeze(-1),
                                  in_=tabs[:, WCELLS - 4:WCELLS, 0:1])

            # gather only the occupied slot range (S_eff <= S); tail calls
            # shrink so the last gather's hidden Q7 write-drain (~27ns/idx)
            # exposes less in the end barrier
            specs = []
            st = 0
            while s_eff - st > 4 * 1024:
                specs.append((st, J))
                st += J
            r = s_eff - st
            if r > 2048:
                specs.append((st, 2048))
                st += 2048
                r -= 2048
            while r > 0:
                sz = min(r, 1024)
                specs.append((st, sz))
                st += sz
                r -= sz

            for c, (st, sz) in enumerate(specs):
                cur = ga[c % 2]
                w_c, o_c = wt[c % 2], ob[c % 2]
                nc.scalar.dma_start(out=w_c[:, 0:sz], in_=wi[:, st:st + sz])
                nc.gpsimd.ap_gather(
                    out_ap=cur[:, 0:sz, :], in_ap=tabs[:],
                    idxs_ap=idxt[:, st // 16:(st + sz) // 16],
                    channels=128, num_elems=WCELLS, d=2, num_idxs=sz)
                nc.vector.tensor_tensor(
                    out=cur[:, 0:sz, :], in0=cur[:, 0:sz, :],
                    in1=w_c[:, 0:sz].unsqueeze(-1).to_broadcast([128, sz, 2]),
                    op=OP.mult)
                for t in range(2 * sz // CHUNK):
                    ps = ppool.tile([16, CHUNK], F32, name="ps")
                    nc.tensor.matmul(
                        ps[:], ones[:],
                        cur[:, t * (CHUNK // 2):(t + 1) * (CHUNK // 2), :])
                    if t % 2 == 0:
                        nc.vector.tensor_copy(
                            out=o_c[:, t * CHUNK:(t + 1) * CHUNK], in_=ps[:])
                    else:
                        nc.scalar.activation(
                            out=o_c[:, t * CHUNK:(t + 1) * CHUNK], in_=ps[:],
                            func=mybir.ActivationFunctionType.Copy)
                nc.sync.dma_start(out=outo[:, 2 * st:2 * (st + sz)],
                                  in_=o_c[:, 0:2 * sz])
    nc.compile()
    return nc


def _build_table(grid: np.ndarray) -> np.ndarray:
    """[NWIN, 16, WCELLS, 2] f16: [w, 2*co+h, e, s] corner-expanded."""
    gp = np.pad(np.asarray(grid, np.float32),
                ((0, 1), (0, 1), (0, 1), (0, 0)), mode="edge").astype(np.float16)
    sw = np.lib.stride_tricks.sliding_window_view(gp, (2, 2, 2), axis=(0, 1, 2))
    # sw: [128,128,128, 4ch, 2dx, 2dy, 2dz]
    e = sw.transpose(0, 1, 2, 4, 5, 6, 3)          # [ix,iy,iz, dx,dy,dz, ch]
    e = e.reshape(NWIN, WCELLS, QC, 2, 2)          # [w, e, co, h, s]
    tab = e.transpose(0, 2, 3, 1, 4)               # [w, co, h, e, s]
    return np.ascontiguousarray(tab, dtype=np.float16).reshape(
        NWIN, 16, WCELLS, 2)


def kernel(x: np.ndarray, grid: np.ndarray) -> np.ndarray:
    global LAST_RESULT, _NC_CACHE
    x = np.asarray(x, dtype=np.float32)
    n = x.shape[0]

    p = np.clip(x * np.float32(NBINS), 0.0, np.float32(NBINS - 1))
    i0 = np.floor(p).astype(np.int32)
    f = p - i0.astype(np.float32)
    cell = (i0[:, 0] * NBINS + i0[:, 1]) * NBINS + i0[:, 2]
    win = cell >> 15
    local = (cell & (WCELLS - 1)).astype(np.int16)

    # 8 weights per point, co = dx*4+dy*2+dz
    wx = np.stack([1.0 - f[:, 0], f[:, 0]], 1)     # [N,2]
    wy = np.stack([1.0 - f[:, 1], f[:, 1]], 1)
    wz = np.stack([1.0 - f[:, 2], f[:, 2]], 1)
    w8 = (wx[:, :, None, None] * wy[:, None, :, None] *
          wz[:, None, None, :]).reshape(n, 8).astype(np.float16)

    order = np.argsort(win, kind="stable")
    counts = np.bincount(win, minlength=NWIN)
    if counts.max() > S:
        raise RuntimeError(f"window overflow: {counts.max()} > {S}")
    offs = np.zeros(NWIN + 1, dtype=np.int64)
    np.cumsum(counts, out=offs[1:])
    assign = np.empty((NWIN, S), dtype=np.int64)
    for w in range(NWIN):
        pts = order[offs[w]:offs[w + 1]]
        pad = pts[0] if len(pts) else order[0]
        assign[w, :len(pts)] = pts
        assign[w, len(pts):] = pad

    s_eff = min(S, (int(counts.max()) + 1023) // 1024 * 1024)

    tab_all = _build_table(grid)                    # [NWIN, 16, WCELLS, 2]

    onesi = np.zeros((128, 16), dtype=np.float16)
    for k in range(QC):
        for co in range(8):
            for h in range(2):
                onesi[16 * k + 2 * co + h, 2 * k + h] = 1.0

    in_maps = []
    for nn in range(CORES):
        sel = assign[QC * nn:QC * nn + QC]          # [8, S]
        # idx: [16k + j%16, j//16]
        loc = local[sel]                            # [8, S]
        idxn = np.ascontiguousarray(
            loc.reshape(QC, S // 16, 16).transpose(0, 2, 1).reshape(128, S // 16))
        # weights: [16k + 2co + h, j]
        wn = w8[sel]                                # [8, S, 8]
        wn = wn.transpose(0, 2, 1)                  # [8k, 8co, S]
        wn = np.repeat(wn[:, :, None, :], 2, axis=2)  # [8k, 8co, 2h, S]
        wn = np.ascontiguousarray(wn.reshape(128, S))
        tabn = np.ascontiguousarray(
            tab_all[QC * nn:QC * nn + QC].reshape(128, WCELLS, 2))
        in_maps.append({"tabi": tabn, "wi": wn, "idxi": idxn, "onesi": onesi})

    if _NC_CACHE is None:
        _NC_CACHE = _build(s_eff)
    res = run_bass_kernel_spmd(_NC_CACHE, in_maps, core_ids=list(range(CORES)))
    LAST_RESULT = res

    outp = np.empty((n, OUTC), dtype=np.float32)
    for nn in range(CORES):
        o = np.asarray(res.results[nn]["outo"], dtype=np.float32)
        vals = o.reshape(QC, 2, S, 2).transpose(0, 2, 1, 3).reshape(QC, S, OUTC)
        for k in range(QC):
            w = QC * nn + k
            cnt = int(counts[w])
            outp[assign[w, :cnt]] = vals[k, :cnt]
    return outp
